# revision 16
# baseline (speedup 1.0000x reference)
"""Embedding lookup (weight[input_ids]) on 8 Trainium2 NeuronCores.

Strategy: data-parallel over tokens. The 4x2048=8192 token ids are split
into 8 shards of 1024 tokens; every core holds the full [32000, 128] f32
table in HBM, pulls its 1024 rows (512 B each) from HBM into SBUF with
the SWDGE dma_gather instruction, and writes the SBUF block back to its
output shard with a single SWDGE kv_writeback (ctx_idx=0, batch=8,
d_head=128, ncn=n_ctx=128 -- degenerate KV-cache shape that is exactly
a contiguous [128 partitions x 4 KiB] SBUF->HBM store, with partition
p's 8 token-rows as its 8 batch blocks).

Token->SBUF placement is chosen on the host so the store is contiguous:
gather position j = b*128+p handles token p*8+b, which lands token t's
row at SBUF [partition t//8, block t%8].  Partition p then holds tokens
p*8..p*8+7 back to back, so the writeback of partition p's 4 KiB at
byte offset p*4096 of the shard reproduces natural token order.

Pipeline (per core):
  SP  : ids DMA (HWDGE, all 64 wrapped idx columns) -> drain (on HW the
        drain waits the HWDGE FIFO, i.e. data landed) -> engine-sem
        handoff to Pool (skips the ~900ns DMA-sem propagation).
  Pool: gpsimd 'attnmlp' ucode library load and the ctx_idx=0 memset
        (both overlap the ids DMA), then the single 1024-row gather as
        prepare_only on SWDGE queue 0 followed immediately by
        trigger_dma -- the prepare+trigger path skips the 650ns DGE->DMA
        handoff delay of a directly-fired SWDGE op.  Then a Pool drain
        (dge_drain ucode: waits the SWDGE queue drained, i.e. the
        gather's data landed in SBUF; at this point the queue holds only
        the already-triggered gather), then the kv_writeback store is
        prepared and triggered.  Its descriptor generation overlaps the
        gather's DMA transfer, so the store transfer fires the moment
        the DMA engines free up.
No DMA-completion semaphore is waited on anywhere; the ones that exist
(every DMA must carry one for the DGE descriptor encoding) fire
mid-kernel where they are hidden -- except the store's own, which is the
kernel's final event.  The framework preamble is trimmed as in previous
revisions: const-memsets, the entry all-engine barrier and its drains,
and the inter-block branches are stripped (all cross-engine ordering is
carried by explicit semaphores, which the runtime resets between
executions); the exit per-engine drains are kept as the completion
guarantee (Pool's exit dge_drain waits the triggered writeback).

TimelineSim (production cost model) estimate: ~4.55us per core, down
from 6.82us: ids handoff 0.69us -> gather descriptor-gen on Pool
0.73-2.07us (994ns SWDGE fixed + 0.34ns/row) -> trigger 2.10us ->
gather transfer 2.10-3.56us (1024 descriptors x 512B at the modeled
360 B/ns) -> store transfer 3.56-3.65us (the kv_writeback's 65
stripe descriptors of 512B) -> store DMA-sem propagation 0.90us ->
4.55us.
"""

import numpy as np

VOCAB = 32000
EMBED = 128
N_CORES = 8
B, S = 4, 2048
N = B * S                 # 8192 tokens total
NPC = N // N_CORES        # 1024 tokens per core
BLK = NPC // 128          # 8 blocks of 128 gather positions
IDXW = NPC // 16          # 64 idx columns in the wrapped idx layout

_NC_CACHE = {}
_STORE_MODE = "kv"   # flips to "hwdge" if the kv path fails in this env


def build_nc(store_mode=None):
    """Build the per-core Bass program (identical on all 8 cores).

    store_mode "kv" (default): the SBUF->HBM store is a single degenerate
    kv_writeback (65 stripe descriptors of 512B, ~92ns modeled).  "hwdge":
    plain
    HWDGE DMACopy store on SP released by a Pool drain + engine sem
    (~5.9us total) -- a conservative fallback using only the same
    instructions as the original baseline.
    """
    from contextlib import ExitStack

    import concourse.bacc as bacc
    import concourse.mybir as mybir
    from concourse import library_config
    from concourse.bass import AP

    if store_mode is None:
        store_mode = _STORE_MODE
    kv = store_mode == "kv"

    nc = bacc.Bacc("TRN2", target_bir_lowering=False, num_devices=N_CORES,
                   num_swdge_queues=1)

    ids_d = nc.dram_tensor("ids", [128, IDXW], mybir.dt.int16,
                           kind="ExternalInput")
    w_d = nc.dram_tensor("weight", [VOCAB, EMBED], mybir.dt.float32,
                         kind="ExternalInput")
    out_d = nc.dram_tensor("out", [NPC, EMBED], mybir.dt.float32,
                           kind="ExternalOutput")

    with ExitStack() as stack:
        block = stack.enter_context(nc.Block())
        ids_sem = stack.enter_context(nc.semaphore("ids_sem"))
        ids_dma_sem = stack.enter_context(nc.semaphore("ids_dma_sem"))
        prep_sem = stack.enter_context(nc.semaphore("prep_sem"))
        dma_sem = stack.enter_context(nc.semaphore("dma_sem"))
        kprep_sem = stack.enter_context(nc.semaphore("kprep_sem"))
        kdma_sem = stack.enter_context(nc.semaphore("kdma_sem"))
        gd_sem = stack.enter_context(nc.semaphore("gd_sem"))
        st_sem = stack.enter_context(nc.semaphore("st_sem"))
        idx_t = stack.enter_context(
            nc.sbuf_tensor("idx_t", [128, IDXW], mybir.dt.int16))
        gath_t = stack.enter_context(
            nc.sbuf_tensor("gath_t", [128, NPC], mybir.dt.float32))
        ctx_t = stack.enter_context(
            nc.sbuf_tensor("ctx_t", [128, BLK], mybir.dt.int32))

        # kv_writeback APs (strides in elements).  Shape chosen to
        # minimize the modeled stripe-descriptor transfer time
        # (batch*d_head/16+1 descriptors x ncn*4 bytes): batch=8,
        # d_head=128, ncn=n_ctx=128 -> 65 descriptors of exactly 512B,
        # 92ns modeled (vs 102ns for the batch=1, ncn=1024 shape).
        #   in  4-D over gath_t, partitions first (SBUF dim0 is the
        #        partition dim): [128 partitions, dho=1, batch=8,
        #        ncn=128]; partition p's 8 token-rows are its 8 batch
        #        blocks.  The ucode/interp view is (128, dho, batch,
        #        ncn).
        #   out [batch=8, d_head_inner=1, d_head_outer=128, n_ctx=128]
        #        over the flat [1024,128]f32 shard: batch stride 128
        #        elements (one row), partition stride 1024 elements ->
        #        token p*8+b lands at row p*8+b, ctx contiguous.
        g_ap = gath_t[:]
        in4 = AP(g_ap.tensor, g_ap.offset,
                 [[NPC, 128], [NPC, 1], [EMBED, BLK], [1, EMBED]])
        o_ap = out_d.ap()
        out4 = AP(o_ap.tensor, o_ap.offset,
                  [[EMBED, BLK], [NPC * 128, 1], [NPC, 128], [1, EMBED]])

        @block.gpsimd
        def _(g):
            g.load_library(library_config.attnmlp)
            npc_reg = g.to_reg(NPC)
            if kv:
                g.memset(ctx_t[:], 0)
            g.wait_ge(ids_sem, 16)
            g.dma_gather(
                gath_t[:].rearrange("p (b e) -> p b e", e=EMBED),
                w_d.ap(),
                idx_t[:],
                NPC,          # num_idxs
                npc_reg,      # num_idxs_reg (all indices valid)
                EMBED,        # elem_size (one table row)
                prepare_only=True,
                sem=dma_sem,
            ).then_inc(prep_sem, 1)
            g.wait_ge(prep_sem, 1)
            g.trigger_dma(1)
            # dge_drain (drain.cpp/dge_ring_metadata.cpp): pops each
            # TRIGGERED ring entry and waits its DMA completion semaphore
            # -- i.e. on HW this blocks until the gather's data landed.
            # It must run while the ring holds no UNTRIGGERED entries
            # (those raise an illegal-instruction error), hence the kv
            # store is prepared strictly after this drain.
            if kv:
                g.drain()
                g.kv_writeback(out4, in4, ctx_t[:],
                               prepare_only=True, sem=kdma_sem
                               ).then_inc(kprep_sem, 1)
                g.wait_ge(kprep_sem, 1)
                g.trigger_dma(1)
            else:
                g.drain().then_inc(gd_sem, 16)

        @block.sync
        def _(sp):
            sp.dma_start(idx_t[:], ids_d.ap()).then_inc(ids_dma_sem, 16)
            sp.drain().then_inc(ids_sem, 16)
            if not kv:
                sp.wait_ge(gd_sem, 16)
                sp.dma_start(
                    out_d.ap().rearrange("(r k) e -> r (k e)", r=128),
                    gath_t[:],
                ).then_inc(st_sem, 16)

    # Strip the framework preamble this kernel doesn't need: const-memset
    # tiles nothing reads, the entry all-engine barrier and its drains
    # (all cross-engine ordering here is carried by explicit semaphores),
    # and the exit barrier's EventSemaphore exchange.  The exit per-engine
    # Drains are kept: they are the completion guarantee.  Body blocks are
    # merged afterwards, so only the entry block's preamble is filtered.
    import concourse.mybir as mybir

    blk = nc.m.functions[0].blocks[0]
    blk.instructions = [
        i for i in blk.instructions
        if not (isinstance(i, mybir.InstMemset) and i.outs
                and str(getattr(i.outs[0], "memref", "")).startswith("const-"))
        and not isinstance(i, (mybir.InstDrain, mybir.InstEventSemaphore))
    ]
    end_blk = nc.m.functions[0].blocks[-1]
    end_blk.instructions = [
        i for i in end_blk.instructions
        if not isinstance(i, mybir.InstEventSemaphore)
    ]
    # Merge the per-engine body blocks into one branchless block (the
    # inter-block branch hops sat on the ids critical path).
    f = nc.m.functions[0]
    merged = []
    for b in f.blocks:
        for ins in b.instructions:
            if isinstance(ins, mybir.InstUnconditionalBranch):
                continue
            merged.append(ins)
    f.blocks[0].instructions = merged
    del f.blocks[1:]

    nc.compile()
    return nc


def _get_nc():
    if _STORE_MODE not in _NC_CACHE:
        _NC_CACHE[_STORE_MODE] = build_nc(_STORE_MODE)
    return _NC_CACHE[_STORE_MODE]


def _wrap16(vals):
    """[n] -> [128, n//16] int16 in the SWDGE wrapped idx layout: value j at
    partition j%16, column j//16, replicated to all 8 gpsimd cores (16
    partitions each)."""
    w = vals.reshape(-1, 16).T                           # [16, n/16]
    return np.tile(w, (8, 1)).astype(np.int16)           # [128, n/16]


def prep_ids(ids_flat):
    """Per-core wrapped int16 idx arrays: gather position b*128+p looks up
    token p*8+b, so SBUF partition p receives its 8 token rows
    back-to-back and the store is a flat contiguous writeback."""
    per_core = []
    for c in range(N_CORES):
        shard = ids_flat[c * NPC:(c + 1) * NPC]
        pos = shard.reshape(128, BLK).T.reshape(-1)      # pos[b*128+p]
        per_core.append(np.ascontiguousarray(_wrap16(pos)))
    return per_core


def run_spmd(inputs, trace=False, nc=None):
    """Returns (output [4,2048,128] f32, BassKernelResults)."""
    from concourse.bass_utils import run_bass_kernel_spmd

    ids = np.asarray(inputs["input_ids"]).reshape(-1).astype(np.int64)
    w = np.ascontiguousarray(np.asarray(inputs["weight"], dtype=np.float32))
    assert ids.shape == (N,) and w.shape == (VOCAB, EMBED)

    in_maps = [{"ids": ids_c, "weight": w} for ids_c in prep_ids(ids)]
    res = run_bass_kernel_spmd(
        nc if nc is not None else _get_nc(),
        in_maps,
        core_ids=list(range(N_CORES)),
        trace=trace,
    )
    shards = [r["out"] for r in res.results]
    out = np.concatenate(shards, axis=0).reshape(B, S, EMBED)
    return np.ascontiguousarray(out.astype(np.float32)), res


def kernel(**inputs):
    global _STORE_MODE
    try:
        out, _ = run_spmd(inputs, trace=False)
        return out
    except Exception:
        if _STORE_MODE != "kv":
            raise
        # Insurance: if the kv_writeback path fails to compile/run in this
        # environment, fall back to the plain HWDGE-store variant.
        _STORE_MODE = "hwdge"
        _NC_CACHE.clear()
        out, _ = run_spmd(inputs, trace=False)
        return out
